# revision 20
# baseline (speedup 1.0000x reference)
"""Trainium2 Bass kernel for nn_GroupPointEncoder.

Reference computation (G=4, B=8, N=2048, F=128):
  std = 2 or 4 per point by label class
  coords = [point_coord, (point_coord + noise*std)[1:]]           # [G,B,N,3]
  normed = (coords - low) / (high - low)
  pe     = interleaved sin/cos embedding, (y,x,z) order            # [G,B,N,384]
  h      = relu(pe @ W1.T + b1)                                    # [G,B,N,512]
  pos    = h @ W2.T + b2                                           # [G,B,N,256]
  query  = label_weight[labels] + pos
  out    = concat([query_pos, query], -1).reshape(G*B, N, 512)

Sharding: data-parallel over the G*B=32 (g,b) pairs, 4 per core, 8 cores.
Each core computes its 4*2048=8192 points' `query` half on device; the
query_pos half is a passthrough assembled on the host.

Device pipeline. Phase stage runs on 1024-point macro-tiles (halves op and
semaphore overhead); matmul stages run on 512-point tiles (PSUM bank size):
  bc[128,3,2T]    = DMA partition-broadcast of prescaled coords (f32)
  k  (int32)      = round(bc*s/2pi + b/2pi)          GpSimd tensor_scalar
  r               = bc - k*(2pi/s)  (Cody-Waite)     DVE custom op
  pe (bf16)       = Sin(r*s + b)                     1 ACT op, arg in [-pi,pi]
  per 512 tile:
    h (bf16)      = relu(W1p @ pe + b1)              12 bf16 matmuls,
                                                     relu split ACT/DVE
    q  (f32)      = W2 @ h + onehot.T@(lab_w+b2)     10 bf16 matmuls,
                                                     PSUM->SBUF copy ACT/DVE
  one output DMA per macro-tile
"""
import sys
import math

sys.path.insert(0, "/opt/trn_rl_repo")

import numpy as np
import ml_dtypes
from contextlib import ExitStack

import concourse.bass as bass
import concourse.tile as tile
from concourse import bacc, library_config, mybir
from concourse.bass_utils import run_bass_kernel_spmd

# problem constants (hardcoded per contract)
G, B, N, F = 4, 8, 2048, 128
NCORES = 8
BPC = B * G // NCORES          # 4 (g,b) pairs per core
NPTS = BPC * N                 # 8192 points per core
T = 512                        # points per matmul tile (PSUM bank)
M = 2 * T                      # points per phase macro-tile
NM = NPTS // M                 # 8 macro-tiles
TWO_PI = 2.0 * math.pi
F32 = mybir.dt.float32
BF16 = mybir.dt.bfloat16
I32 = mybir.dt.int32
BF16_NP = ml_dtypes.bfloat16

_CACHE = {}


def _build_program():
    nc = bacc.Bacc("TRN2", target_bir_lowering=False, debug=False, num_devices=NCORES)

    pc_d = nc.dram_tensor("pc", [NM, 1, 3, M], F32, kind="ExternalInput").ap()
    oh_d = nc.dram_tensor("oh", [NM, 10, M], BF16, kind="ExternalInput").ap()
    w1t_d = nc.dram_tensor("w1t", [3, 128, 512], BF16, kind="ExternalInput").ap()
    w2t_d = nc.dram_tensor("w2t", [4, 128, 256], BF16, kind="ExternalInput").ap()
    lwb_d = nc.dram_tensor("lwb", [10, 256], BF16, kind="ExternalInput").ap()
    # packed per-partition consts: svec bvec sdiv bdiv ivhi ivmid b1c[4]
    cst_d = nc.dram_tensor("cst", [128, 10], F32, kind="ExternalInput").ap()
    q_d = nc.dram_tensor("q", [128, 2, NPTS], F32, kind="ExternalOutput").ap()

    with tile.TileContext(nc) as tc, ExitStack() as ctx:
        cpool = ctx.enter_context(tc.tile_pool(name="consts", bufs=1))
        wpool = ctx.enter_context(tc.tile_pool(name="weights", bufs=1))
        io = ctx.enter_context(tc.tile_pool(name="io", bufs=3))
        bcp = ctx.enter_context(tc.tile_pool(name="bcp", bufs=3))
        work = ctx.enter_context(tc.tile_pool(name="work", bufs=2))
        hpool = ctx.enter_context(tc.tile_pool(name="hpool", bufs=3))
        qsp = ctx.enter_context(tc.tile_pool(name="qsp", bufs=2))
        psum_h = ctx.enter_context(tc.tile_pool(name="ph", bufs=3, space="PSUM"))
        psum_q = ctx.enter_context(tc.tile_pool(name="pq", bufs=1, space="PSUM"))

        bc_tiles, oh_tiles, out_pend = {}, {}, {}

        def _prefetch(t):
            if t >= NM:
                return
            bc_ = bcp.tile([128, 3, M], F32, tag="bc")
            nc.sync.dma_start(bc_[:], pc_d[t].to_broadcast((128, 3, M)))
            oh_ = io.tile([10, M], BF16, tag="oh_t")
            nc.sync.dma_start(oh_[:], oh_d[t])
            bc_tiles[t], oh_tiles[t] = bc_, oh_

        # DMA issue order = first-use order: tile 0 inputs, consts, W1,
        # tile 1 inputs, W2 + label table
        _prefetch(0)
        cst = cpool.tile([128, 10], F32)
        nc.sync.dma_start(cst[:], cst_d[:])
        svec, bvec = cst[:, 0:1], cst[:, 1:2]
        sdiv, bdiv = cst[:, 2:3], cst[:, 3:4]
        ivhi, ivmid = cst[:, 4:5], cst[:, 5:6]
        b1c = cst[:, 6:10]
        w1t = []
        for kk in range(3):
            w = wpool.tile([128, 512], BF16, name=f"w1t{kk}", tag=f"w1t{kk}")
            nc.sync.dma_start(w[:], w1t_d[kk])
            w1t.append(w)
        _prefetch(1)
        w2t = []
        for kk in range(4):
            w = wpool.tile([128, 256], BF16, name=f"w2t{kk}", tag=f"w2t{kk}")
            nc.sync.dma_start(w[:], w2t_d[kk])
            w2t.append(w)
        lwb = cpool.tile([10, 256], BF16)
        nc.sync.dma_start(lwb[:], lwb_d[:])

        for t in range(NM):
            _prefetch(t + 2)
            if t - 1 in out_pend:
                nc.sync.dma_start(
                    q_d[:, :, (t - 1) * M : t * M], out_pend.pop(t - 1)[:]
                )
            bc = bc_tiles.pop(t)
            oh_t = oh_tiles.pop(t)

            # ---- stage 1: range reduction.  k = round(bc*s/2pi + b/2pi);
            # r = bc - k*(2pi/s) via Cody-Waite; then s*r + b lands in [-pi,pi]
            bcf = bc[:].rearrange("p a b -> p (a b)")
            k = work.tile([128, 3 * M], I32, tag="k")
            nc.gpsimd.tensor_scalar(
                k[:], bcf, sdiv, bdiv,
                op0=mybir.AluOpType.mult, op1=mybir.AluOpType.add,
            )
            r = work.tile([128, 3 * M], F32, tag="r")
            nc.vector.cody_waite_cascade(r[:], bcf, k[:], ivhi, ivmid, 0.0)

            # ---- stage 2: pe = sin(s*r + b)  (rows 0:64 sin, 64:128 cos)
            pe = work.tile([128, 3, M], BF16, tag="pe")
            nc.scalar.activation(
                pe[:].rearrange("p a b -> p (a b)"), r[:],
                mybir.ActivationFunctionType.Sin,
                bias=bvec, scale=svec,
            )

            qs = qsp.tile([128, 2, M], F32, tag="qs")
            for it in range(2):
                pcol = slice(it * T, (it + 1) * T)

                # ---- stage 3: h = relu(W1p @ pe + b1), feature-major
                # two PSUM half-tiles; relu split between ACT and DVE
                h = hpool.tile([128, 4, T], BF16, tag="h")
                for half in range(2):
                    hp = psum_h.tile([128, 2, T], F32, tag="hp")
                    for m2 in range(2):
                        m = 2 * half + m2
                        for kk in range(3):
                            nc.tensor.matmul(
                                hp[:, m2, :],
                                w1t[kk][:, m * 128 : (m + 1) * 128],
                                pe[:, kk, pcol],
                                start=(kk == 0),
                                stop=(kk == 2),
                            )
                        if m % 2 == 1:
                            nc.vector.tensor_scalar(
                                h[:, m, :], hp[:, m2, :], b1c[:, m : m + 1], 0.0,
                                op0=mybir.AluOpType.add, op1=mybir.AluOpType.max,
                            )
                        else:
                            nc.scalar.activation(
                                h[:, m, :],
                                hp[:, m2, :],
                                mybir.ActivationFunctionType.Relu,
                                bias=b1c[:, m : m + 1],
                            )

                # ---- stage 4: q = W2 @ h + onehot-gather, feature-major
                qp = psum_q.tile([128, 2, T], F32, tag="qp")
                for mp in range(2):
                    for kk in range(4):
                        nc.tensor.matmul(
                            qp[:, mp, :],
                            w2t[kk][:, mp * 128 : (mp + 1) * 128],
                            h[:, kk, :],
                            start=(kk == 0),
                            stop=False,
                        )
                    nc.tensor.matmul(
                        qp[:, mp, :],
                        lwb[:, mp * 128 : (mp + 1) * 128],
                        oh_t[:, pcol],
                        start=False,
                        stop=True,
                    )
                # PSUM -> SBUF staging copy, alternating ACT / DVE
                if it == 0:
                    nc.scalar.activation(
                        qs[:, :, pcol], qp[:],
                        mybir.ActivationFunctionType.Copy,
                    )
                else:
                    nc.vector.tensor_copy(qs[:, :, pcol], qp[:])
            out_pend[t] = qs
        nc.sync.dma_start(q_d[:, :, (NM - 1) * M :], out_pend.pop(NM - 1)[:])

    nc.compile()
    return nc


def _host_prep(point_coord, labels, pc_range, noise, label_weight, W1, b1, W2, b2):
    """Build the per-core input maps (host-side sharding + weight prep)."""
    pc32 = np.asarray(point_coord, np.float32)
    lab = np.asarray(labels)
    noi = np.asarray(noise, np.float32)
    rng = np.asarray(pc_range, np.float32)

    small = (lab == 0) | (lab >= 6)
    std = np.where(small, 2.0, 4.0).astype(np.float32)            # [B,N]
    coords = pc32[None] + noi * std[None, :, :, None]             # [G,B,N,3]
    coords[0] = pc32                                              # group 0 originals
    low, high = rng[:3], rng[3:]
    pcs = (coords - low) / (high - low) * np.float32(TWO_PI)      # [G,B,N,3]
    pcs = pcs[..., [1, 0, 2]]   # reference concatenates pe in (y,x,z) order
    onehot = np.eye(10, dtype=np.float32)[np.asarray(lab, np.int64)]  # [B,N,10]

    # feature permutation: kernel row c*128+j -> ref feature c*128+2j (sin),
    # row c*128+64+j -> c*128+2j+1 (cos)
    perm = np.empty(3 * F, np.int64)
    for c in range(3):
        for j in range(64):
            perm[c * 128 + j] = c * 128 + 2 * j
            perm[c * 128 + 64 + j] = c * 128 + 2 * j + 1
    w1p = np.ascontiguousarray(np.asarray(W1, np.float32)[:, perm].T)  # [384,512]
    w2t = np.ascontiguousarray(np.asarray(W2, np.float32).T)           # [512,256]
    lwb = np.asarray(label_weight, np.float32) + np.asarray(b2, np.float32)[None]
    b1c = np.ascontiguousarray(np.asarray(b1, np.float32).reshape(4, 128).T)

    j64 = np.arange(64, dtype=np.float64)
    s64 = 10000.0 ** (-j64 / 64.0)
    s128 = np.concatenate([s64, s64])
    b128 = np.concatenate([np.zeros(64), np.full(64, np.pi / 2)])
    inv = 2 * np.pi / s128                                         # f64
    ivhi = inv.astype(np.float32).view(np.uint32) & np.uint32(0xFFFFE000)
    ivhi = ivhi.view(np.float32)          # 10 explicit mantissa bits: k*ivhi exact
    ivmid = (inv - ivhi.astype(np.float64)).astype(np.float32)

    cst = np.empty((128, 10), np.float32)
    cst[:, 0] = s128
    cst[:, 1] = b128
    cst[:, 2] = s128 / (2 * np.pi)
    cst[:, 3] = b128 / (2 * np.pi)
    cst[:, 4] = ivhi
    cst[:, 5] = ivmid
    cst[:, 6:10] = b1c

    shared = {
        "w1t": w1p.astype(BF16_NP).reshape(3, 128, 512),
        "w2t": w2t.astype(BF16_NP).reshape(4, 128, 256),
        "lwb": np.ascontiguousarray(lwb.astype(BF16_NP)),
        "cst": cst,
    }

    in_maps = []
    for core in range(NCORES):
        g = core // 2
        b0 = 4 * (core % 2)
        # [4b, N, 3] -> [3, NPTS] -> [3, NM, M] -> [NM, 3, M]
        pcc = pcs[g, b0 : b0 + 4].reshape(NPTS, 3).T
        pcc = np.ascontiguousarray(pcc.reshape(3, NM, M).transpose(1, 0, 2)).reshape(
            NM, 1, 3, M
        )
        ohc = onehot[b0 : b0 + 4].reshape(NPTS, 10).T
        ohc = np.ascontiguousarray(
            ohc.reshape(10, NM, M).transpose(1, 0, 2).astype(BF16_NP)
        )
        in_maps.append({"pc": pcc, "oh": ohc, **shared})
    return in_maps


def _get_nc():
    if "nc" not in _CACHE:
        _CACHE["nc"] = _build_program()
    return _CACHE["nc"]


def _run_device(in_maps, trace=False, **kw):
    nc = _get_nc()
    return run_bass_kernel_spmd(nc, in_maps, list(range(NCORES)), trace=trace, **kw)


def kernel(point_coord, labels, pc_range, noise, query_pos, label_weight, W1, b1, W2, b2):
    in_maps = _host_prep(
        point_coord, labels, pc_range, noise, label_weight, W1, b1, W2, b2
    )
    res = _run_device(in_maps)

    qp = np.asarray(query_pos, np.float32)
    out = np.empty((G * B, N, 4 * F), np.float32)
    out[:, :, : 2 * F] = qp.reshape(G * B, N, 2 * F)
    for core in range(NCORES):
        q3 = res.results[core]["q"]                      # [128, 2, NPTS]
        q = q3.transpose(1, 0, 2).reshape(2 * F, BPC, N)  # [256, 4, N]
        out[4 * core : 4 * core + 4, :, 2 * F :] = q.transpose(1, 2, 0)
    return out
